# revision 3
# baseline (speedup 1.0000x reference)
"""Depthwise causal conv1d (K=4, dilation=1) on 8 TRN2 NeuronCores.

Reference: x [B=8, T=4096, C=1024] f32, W [4, 1, 1024] f32 (WIO layout),
y[b, t, c] = sum_k W[k, 0, c] * x[b, t - 3 + k, c]  (zero left-pad).

Sharding: pure batch data-parallel — core i computes batch i. On the host we
pre-transpose each batch slice to [C, T] (contiguous) so that on-chip the
channel dim sits on SBUF partitions (the per-channel weight becomes a
per-partition scalar operand) and the causal time shifts become free-dim
offsets. The device writes y in [C, T] layout; the host transposes back.

Per-core compute, per channel-group g (8 groups of 128 channels) and time
tile: load x tile [128, TT+3] (3-col halo, zero for t<0). Work is split
between two fp32-exact paths to balance engines (VectorE alone would be
the bottleneck at ~122us vs the ~94us HBM roofline):
 - DVE path: ScalarE seeds y = x3*W3 (per-partition scale), then 3x
   VectorE scalar_tensor_tensor accumulates the other taps.
 - PE path: per 512-col chunk, 4 accumulating matmuls with diagonal
   [128x128] weight matrices (built on-chip from an identity by idle
   GpSimd) shift+scale+sum all taps into PSUM; ScalarE evicts PSUM->SBUF.
Loads go on the HWDGE ring (nc.sync), stores on the SWDGE ring
(nc.gpsimd) so compute-gated stores don't head-of-line-block loads.
"""

import numpy as np

B, T, C = 8, 4096, 1024
KTAPS = 4
HALO = KTAPS - 1
CG = 128  # channels per partition-group
N_CORES = 8
MM_N = 512  # fp32 moving-operand max free dim / one PSUM bank

# module-level stash so test.py can read profiling info
last_results = None


def _build_program(c=C, t=T, tt=2048, xbufs=6, ybufs=6, psbufs=8, pe_mod=(3, 1)):
    import concourse.bass as bass  # noqa: F401
    import concourse.tile as tile
    from concourse import bacc, mybir

    nc = bacc.Bacc(
        "TRN2",
        target_bir_lowering=False,
        debug=False,
        enable_asserts=False,
        num_devices=N_CORES,
    )
    n_groups = c // CG
    n_tt = t // tt
    f32 = mybir.dt.float32
    x_ap = nc.dram_tensor("x_t", [c, t], f32, kind="ExternalInput").ap()
    w_ap = nc.dram_tensor("w_t", [CG, n_groups * KTAPS], f32, kind="ExternalInput").ap()
    eye_ap = nc.dram_tensor("eye", [CG, CG], f32, kind="ExternalInput").ap()
    out_ap = nc.dram_tensor("out", [c, t], f32, kind="ExternalOutput").ap()

    mult = mybir.AluOpType.mult
    add = mybir.AluOpType.add

    tiles = [(g, j) for g in range(n_groups) for j in range(n_tt)]
    pe_tile = {
        (g, j): (pe_mod is not None and (g * n_tt + j) % pe_mod[0] == pe_mod[1])
        for (g, j) in tiles
    }
    pe_groups = sorted({g for (g, j) in tiles if pe_tile[(g, j)]})
    dblk = {g: i * KTAPS * CG for i, g in enumerate(pe_groups)}

    with tile.TileContext(nc) as tc:
        with (
            tc.tile_pool(name="wpool", bufs=1) as wpool,
            tc.tile_pool(name="xpool", bufs=xbufs) as xpool,
            tc.tile_pool(name="ypool", bufs=ybufs) as ypool,
            tc.tile_pool(name="pspool", bufs=psbufs, space="PSUM") as pspool,
        ):
            wt = wpool.tile([CG, n_groups * KTAPS], f32)
            nc.sync.dma_start(wt[:], w_ap[:])
            eye = wpool.tile([CG, CG], f32)
            nc.sync.dma_start(eye[:], eye_ap[:])
            # build diag(W[k, g*CG:(g+1)*CG]) blocks on GpSimd (otherwise idle)
            wd = None
            if pe_groups:
                wd = wpool.tile([CG, len(pe_groups) * KTAPS * CG], f32)
                for g in pe_groups:
                    for k in range(KTAPS):
                        blk = dblk[g] + k * CG
                        nc.gpsimd.tensor_scalar_mul(
                            wd[:, blk : blk + CG],
                            eye[:],
                            wt[:, g * KTAPS + k : g * KTAPS + k + 1],
                        )

            for g, j in tiles:
                r0, r1 = g * CG, (g + 1) * CG
                t0 = j * tt
                xt = xpool.tile([CG, tt + HALO], f32)
                if j == 0:
                    nc.gpsimd.memset(xt[:, 0:HALO], 0.0)
                    nc.sync.dma_start(xt[:, HALO : HALO + tt], x_ap[r0:r1, 0:tt])
                else:
                    nc.sync.dma_start(xt[:], x_ap[r0:r1, t0 - HALO : t0 + tt])
                yt = ypool.tile([CG, tt], f32)
                if pe_tile[(g, j)]:
                    for c0 in range(0, tt, MM_N):
                        ps = pspool.tile([CG, MM_N], f32)
                        for ki, k in enumerate((3, 2, 1, 0)):
                            dcol = dblk[g] + k * CG
                            nc.tensor.matmul(
                                ps[:],
                                wd[:, dcol : dcol + CG],
                                xt[:, c0 + k : c0 + k + MM_N],
                                start=(ki == 0),
                                stop=(ki == KTAPS - 1),
                            )
                        nc.scalar.copy(yt[:, c0 : c0 + MM_N], ps[:])
                else:
                    wcol = g * KTAPS
                    # seed with the last tap on ScalarE (keeps VectorE at 3 ops)
                    nc.scalar.mul(
                        yt[:], xt[:, HALO : HALO + tt], wt[:, wcol + 3 : wcol + 4]
                    )
                    for k in (2, 1, 0):
                        nc.vector.scalar_tensor_tensor(
                            yt[:],
                            xt[:, k : k + tt],
                            wt[:, wcol + k : wcol + k + 1],
                            yt[:],
                            op0=mult,
                            op1=add,
                        )
                nc.gpsimd.dma_start(out_ap[r0:r1, t0 : t0 + tt], yt[:])
    nc.compile()
    return nc


def _prep_weights(W: np.ndarray, c=C) -> np.ndarray:
    # wt[p, g*KTAPS + k] = W[k, 0, g*CG + p]
    n_groups = c // CG
    wk = W.reshape(KTAPS, n_groups, CG)  # [k, g, p]
    return np.ascontiguousarray(wk.transpose(2, 1, 0).reshape(CG, n_groups * KTAPS))


def kernel(x: np.ndarray, W: np.ndarray) -> np.ndarray:
    global last_results
    from concourse.bass_utils import run_bass_kernel_spmd

    x = np.asarray(x, dtype=np.float32)
    W = np.asarray(W, dtype=np.float32)
    assert x.shape == (B, T, C) and W.shape == (KTAPS, 1, C)

    nc = _build_program()
    wt = _prep_weights(W)
    eye = np.eye(CG, dtype=np.float32)
    in_maps = [
        {
            "x_t": np.ascontiguousarray(x[i].T),  # [C, T]
            "w_t": wt,
            "eye": eye,
        }
        for i in range(N_CORES)
    ]
    import os

    trace = bool(os.environ.get("BASS_TRACE"))
    res = run_bass_kernel_spmd(
        nc, in_maps, core_ids=list(range(N_CORES)), trace=trace
    )
    last_results = res
    y = np.stack([np.asarray(res.results[i]["out"]).T for i in range(N_CORES)])
    return np.ascontiguousarray(y.astype(np.float32))


# revision 7
# speedup vs baseline: 1.1156x; 1.1156x over previous
"""Depthwise causal conv1d (K=4, dilation=1) on 8 TRN2 NeuronCores.

Reference: x [B=8, T=4096, C=1024] f32, W [4, 1, 1024] f32 (WIO layout),
y[b, t, c] = sum_k W[k, 0, c] * x[b, t - 3 + k, c]  (zero left-pad).

Sharding: pure batch data-parallel — core i computes batch i. On the host we
pre-transpose each batch slice to [C, T] (contiguous) so that on-chip the
channel dim sits on SBUF partitions (the per-channel weight becomes a
per-partition scalar operand) and the causal time shifts become free-dim
offsets. The device writes y in [C, T] layout; the host transposes back.

Per-core compute, per channel-group g (8 groups of 128 channels) and time
tile: load x tile [128, TT+3] (3-col halo; x is host-padded with 3 zero
columns so all tiles load uniformly). Work is split between two
fp32-exact paths to balance engines (VectorE alone would be the
bottleneck at ~122us vs the ~94us HBM roofline):
 - DVE path: ScalarE seeds y = x3*W3 (per-partition scale), then 3x
   VectorE scalar_tensor_tensor accumulates the other taps.
 - PE path: per 512-col chunk, 4 accumulating matmuls with diagonal
   [128x128] weight matrices (built on-chip from an identity by ScalarE)
   shift+scale+sum all taps into PSUM; ScalarE evicts PSUM->SBUF.
Loads go on the HWDGE ring (nc.sync), stores on the SWDGE ring
(nc.gpsimd) so compute-gated stores don't head-of-line-block loads.
"""

import numpy as np

B, T, C = 8, 4096, 1024
KTAPS = 4
HALO = KTAPS - 1
CG = 128  # channels per partition-group
N_CORES = 8
MM_N = 512  # fp32 moving-operand max free dim / one PSUM bank

# module-level stash so test.py can read profiling info
last_results = None


def _build_program(c=C, t=T, tt=2048, xbufs=6, ybufs=6, psbufs=8, pe_mod=(3, 1)):
    import concourse.bass as bass  # noqa: F401
    import concourse.tile as tile
    from concourse import bacc, mybir

    nc = bacc.Bacc(
        "TRN2",
        target_bir_lowering=False,
        debug=False,
        enable_asserts=False,
        num_devices=N_CORES,
    )
    n_groups = c // CG
    n_tt = t // tt
    f32 = mybir.dt.float32
    x_ap = nc.dram_tensor("x_t", [c, t + HALO], f32, kind="ExternalInput").ap()
    w_ap = nc.dram_tensor("w_t", [CG, n_groups * KTAPS], f32, kind="ExternalInput").ap()
    eye_ap = nc.dram_tensor("eye", [CG, CG], f32, kind="ExternalInput").ap()
    out_ap = nc.dram_tensor("out", [c, t], f32, kind="ExternalOutput").ap()

    mult = mybir.AluOpType.mult
    add = mybir.AluOpType.add

    tiles = [(g, j) for g in range(n_groups) for j in range(n_tt)]
    pe_tile = {
        (g, j): (pe_mod is not None and (g * n_tt + j) % pe_mod[0] == pe_mod[1])
        for (g, j) in tiles
    }
    pe_groups = sorted({g for (g, j) in tiles if pe_tile[(g, j)]})
    dblk = {g: i * KTAPS * CG for i, g in enumerate(pe_groups)}

    with tile.TileContext(nc) as tc:
        with (
            tc.tile_pool(name="wpool", bufs=1) as wpool,
            tc.tile_pool(name="xpool", bufs=xbufs) as xpool,
            tc.tile_pool(name="ypool", bufs=ybufs) as ypool,
            tc.tile_pool(name="pspool", bufs=psbufs, space="PSUM") as pspool,
        ):
            wt = wpool.tile([CG, n_groups * KTAPS], f32)
            nc.sync.dma_start(wt[:], w_ap[:])
            eye = wpool.tile([CG, CG], f32)
            nc.sync.dma_start(eye[:], eye_ap[:])
            # build diag(W[k, g*CG:(g+1)*CG]) blocks on ScalarE (cheap per-op)
            wd = None
            if pe_groups:
                wd = wpool.tile([CG, len(pe_groups) * KTAPS * CG], f32)
                for g in pe_groups:
                    for k in range(KTAPS):
                        blk = dblk[g] + k * CG
                        nc.scalar.mul(
                            wd[:, blk : blk + CG],
                            eye[:],
                            wt[:, g * KTAPS + k : g * KTAPS + k + 1],
                        )

            for g, j in tiles:
                r0, r1 = g * CG, (g + 1) * CG
                t0 = j * tt
                xt = xpool.tile([CG, tt + HALO], f32)
                # x_t is host-padded: column t0 of x_t == time t0 - HALO
                nc.sync.dma_start(xt[:], x_ap[r0:r1, t0 : t0 + tt + HALO])
                yt = ypool.tile([CG, tt], f32)
                if pe_tile[(g, j)]:
                    for c0 in range(0, tt, MM_N):
                        ps = pspool.tile([CG, MM_N], f32)
                        for ki, k in enumerate((3, 2, 1, 0)):
                            dcol = dblk[g] + k * CG
                            nc.tensor.matmul(
                                ps[:],
                                wd[:, dcol : dcol + CG],
                                xt[:, c0 + k : c0 + k + MM_N],
                                start=(ki == 0),
                                stop=(ki == KTAPS - 1),
                            )
                        nc.scalar.copy(yt[:, c0 : c0 + MM_N], ps[:])
                else:
                    wcol = g * KTAPS
                    # seed with the last tap on ScalarE (keeps VectorE at 3 ops)
                    nc.scalar.mul(
                        yt[:], xt[:, HALO : HALO + tt], wt[:, wcol + 3 : wcol + 4]
                    )
                    for k in (2, 1, 0):
                        nc.vector.scalar_tensor_tensor(
                            yt[:],
                            xt[:, k : k + tt],
                            wt[:, wcol + k : wcol + k + 1],
                            yt[:],
                            op0=mult,
                            op1=add,
                        )
                nc.gpsimd.dma_start(out_ap[r0:r1, t0 : t0 + tt], yt[:])
    nc.compile()
    return nc


def _prep_weights(W: np.ndarray, c=C) -> np.ndarray:
    # wt[p, g*KTAPS + k] = W[k, 0, g*CG + p]
    n_groups = c // CG
    wk = W.reshape(KTAPS, n_groups, CG)  # [k, g, p]
    return np.ascontiguousarray(wk.transpose(2, 1, 0).reshape(CG, n_groups * KTAPS))


def kernel(x: np.ndarray, W: np.ndarray) -> np.ndarray:
    global last_results
    from concourse.bass_utils import run_bass_kernel_spmd

    x = np.asarray(x, dtype=np.float32)
    W = np.asarray(W, dtype=np.float32)
    assert x.shape == (B, T, C) and W.shape == (KTAPS, 1, C)

    nc = _build_program()
    wt = _prep_weights(W)
    eye = np.eye(CG, dtype=np.float32)
    zpad = np.zeros((C, HALO), dtype=np.float32)
    in_maps = [
        {
            # [C, T+HALO], causal zero left-pad baked in
            "x_t": np.ascontiguousarray(np.concatenate([zpad, x[i].T], axis=1)),
            "w_t": wt,
            "eye": eye,
        }
        for i in range(N_CORES)
    ]
    import os

    trace = bool(os.environ.get("BASS_TRACE"))
    res = run_bass_kernel_spmd(
        nc, in_maps, core_ids=list(range(N_CORES)), trace=trace
    )
    last_results = res
    y = np.stack([np.asarray(res.results[i]["out"]).T for i in range(N_CORES)])
    return np.ascontiguousarray(y.astype(np.float32))


# revision 9
# speedup vs baseline: 1.2665x; 1.1353x over previous
"""Depthwise causal conv1d (K=4, dilation=1) on 8 TRN2 NeuronCores.

Reference: x [B=8, T=4096, C=1024] f32, W [4, 1, 1024] f32 (WIO layout),
y[b, t, c] = sum_k W[k, 0, c] * x[b, t - 3 + k, c]  (zero left-pad).

Sharding: pure batch data-parallel — core i computes batch i. On the host we
pre-transpose each batch slice to [C, T] (contiguous) so that on-chip the
channel dim sits on SBUF partitions (the per-channel weight becomes a
per-partition scalar operand) and the causal time shifts become free-dim
offsets. The device writes y in [C, T] layout; the host transposes back.

Per-core compute, per channel-group g (8 groups of 128 channels) and time
tile: load x tile [128, TT+3] (3-col halo; x is host-padded with 3 zero
columns so all tiles load uniformly). Work is split between two
fp32-exact paths to balance engines (VectorE alone would be the
bottleneck at ~122us vs the ~94us HBM roofline):
 - DVE path: ScalarE seeds y = x3*W3 (per-partition scale), then 3x
   VectorE scalar_tensor_tensor accumulates the other taps.
 - PE path: per 512-col chunk, 4 accumulating matmuls with diagonal
   [128x128] weight matrices (built on-chip from an identity by ScalarE)
   shift+scale+sum all taps into PSUM; ScalarE evicts PSUM->SBUF.
Loads go on the HWDGE ring (nc.sync), stores on the SWDGE ring
(nc.gpsimd) so compute-gated stores don't head-of-line-block loads.
"""

import numpy as np

B, T, C = 8, 4096, 1024
KTAPS = 4
HALO = KTAPS - 1
CG = 128  # channels per partition-group
N_CORES = 8
MM_N = 512  # fp32 moving-operand max free dim / one PSUM bank

# module-level stash so test.py can read profiling info
last_results = None


def _build_program(c=C, t=T, tt=2048, xbufs=6, ybufs=6, psbufs=8, pe_mod=(3, 1)):
    import concourse.bass as bass  # noqa: F401
    import concourse.tile as tile
    from concourse import bacc, mybir

    nc = bacc.Bacc(
        "TRN2",
        target_bir_lowering=False,
        debug=False,
        enable_asserts=False,
        num_devices=N_CORES,
    )
    n_groups = c // CG
    n_tt = t // tt
    f32 = mybir.dt.float32
    x_ap = nc.dram_tensor("x_t", [c, t + HALO], f32, kind="ExternalInput").ap()
    w_ap = nc.dram_tensor("w_t", [CG, n_groups * KTAPS], f32, kind="ExternalInput").ap()
    eye_ap = nc.dram_tensor("eye", [CG, CG], f32, kind="ExternalInput").ap()
    out_ap = nc.dram_tensor("out", [c, t], f32, kind="ExternalOutput").ap()

    mult = mybir.AluOpType.mult
    add = mybir.AluOpType.add

    tiles = [(g, j) for g in range(n_groups) for j in range(n_tt)]
    pe_tile = {
        (g, j): (pe_mod is not None and (g * n_tt + j) % pe_mod[0] == pe_mod[1])
        for (g, j) in tiles
    }
    pe_groups = sorted({g for (g, j) in tiles if pe_tile[(g, j)]})
    dblk = {g: i * KTAPS * CG for i, g in enumerate(pe_groups)}

    with tile.TileContext(nc) as tc:
        with (
            tc.tile_pool(name="wpool", bufs=1) as wpool,
            tc.tile_pool(name="xpool", bufs=xbufs) as xpool,
            tc.tile_pool(name="ypool", bufs=ybufs) as ypool,
            tc.tile_pool(name="pspool", bufs=psbufs, space="PSUM") as pspool,
        ):
            wt = wpool.tile([CG, n_groups * KTAPS], f32)
            nc.sync.dma_start(wt[:], w_ap[:])
            eye = wpool.tile([CG, CG], f32)
            nc.sync.dma_start(eye[:], eye_ap[:])
            wd = None
            if pe_groups:
                wd = wpool.tile([CG, len(pe_groups) * KTAPS * CG], f32)
            diag_built = set()

            for g, j in tiles:
                r0, r1 = g * CG, (g + 1) * CG
                t0 = j * tt
                xt = xpool.tile([CG, tt + HALO], f32)
                # x_t is host-padded: column t0 of x_t == time t0 - HALO
                nc.sync.dma_start(xt[:], x_ap[r0:r1, t0 : t0 + tt + HALO])
                yt = ypool.tile([CG, tt], f32)
                if pe_tile[(g, j)]:
                    if g not in diag_built:
                        # build diag(W[k, g*CG:(g+1)*CG]) on ScalarE, lazily so
                        # early-tile seeds aren't stuck behind 20 diag builds
                        diag_built.add(g)
                        for k in range(KTAPS):
                            blk = dblk[g] + k * CG
                            nc.scalar.mul(
                                wd[:, blk : blk + CG],
                                eye[:],
                                wt[:, g * KTAPS + k : g * KTAPS + k + 1],
                            )
                    for c0 in range(0, tt, MM_N):
                        ps = pspool.tile([CG, MM_N], f32)
                        for ki, k in enumerate((3, 2, 1, 0)):
                            dcol = dblk[g] + k * CG
                            nc.tensor.matmul(
                                ps[:],
                                wd[:, dcol : dcol + CG],
                                xt[:, c0 + k : c0 + k + MM_N],
                                start=(ki == 0),
                                stop=(ki == KTAPS - 1),
                            )
                        nc.scalar.copy(yt[:, c0 : c0 + MM_N], ps[:])
                else:
                    wcol = g * KTAPS
                    # seed with the last tap on ScalarE (keeps VectorE at 3 ops)
                    nc.scalar.mul(
                        yt[:], xt[:, HALO : HALO + tt], wt[:, wcol + 3 : wcol + 4]
                    )
                    for k in (2, 1, 0):
                        nc.vector.scalar_tensor_tensor(
                            yt[:],
                            xt[:, k : k + tt],
                            wt[:, wcol + k : wcol + k + 1],
                            yt[:],
                            op0=mult,
                            op1=add,
                        )
                nc.gpsimd.dma_start(out_ap[r0:r1, t0 : t0 + tt], yt[:])
    nc.compile()
    return nc


def _prep_weights(W: np.ndarray, c=C) -> np.ndarray:
    # wt[p, g*KTAPS + k] = W[k, 0, g*CG + p]
    n_groups = c // CG
    wk = W.reshape(KTAPS, n_groups, CG)  # [k, g, p]
    return np.ascontiguousarray(wk.transpose(2, 1, 0).reshape(CG, n_groups * KTAPS))


def kernel(x: np.ndarray, W: np.ndarray) -> np.ndarray:
    global last_results
    from concourse.bass_utils import run_bass_kernel_spmd

    x = np.asarray(x, dtype=np.float32)
    W = np.asarray(W, dtype=np.float32)
    assert x.shape == (B, T, C) and W.shape == (KTAPS, 1, C)

    nc = _build_program()
    wt = _prep_weights(W)
    eye = np.eye(CG, dtype=np.float32)
    zpad = np.zeros((C, HALO), dtype=np.float32)
    in_maps = [
        {
            # [C, T+HALO], causal zero left-pad baked in
            "x_t": np.ascontiguousarray(np.concatenate([zpad, x[i].T], axis=1)),
            "w_t": wt,
            "eye": eye,
        }
        for i in range(N_CORES)
    ]
    import os

    trace = bool(os.environ.get("BASS_TRACE"))
    res = run_bass_kernel_spmd(
        nc, in_maps, core_ids=list(range(N_CORES)), trace=trace
    )
    last_results = res
    y = np.stack([np.asarray(res.results[i]["out"]).T for i in range(N_CORES)])
    return np.ascontiguousarray(y.astype(np.float32))
